# revision 12
# baseline (speedup 1.0000x reference)
"""Trainium2 Bass kernel for nn_CCIM (dot-product intervention / CCIM block).

Reference computation (B=1024, K=256, D=1024, P=768):
    q = jf @ Wq                      [B, P]
    k = conf @ Wk                    [K, P]
    s = (q @ k.T) / 32               [B, K]
    a = softmax(s, axis=-1)          [B, K]
    out = jf + a @ (conf * prior)    [B, D]

Distribution: data-parallel over B across 8 NeuronCores (128 rows each);
conf/prior/Wq/Wk replicated on every core; no collectives.

Per-core algorithm (matmuls in bf16 with fp32 PSUM accumulation; jf stays
exact fp32 through to the final add, so bf16 rounding only perturbs the small
g_z correction term):
  weights stream in as bf16 via SWDGE cast-DMA (cast happens in the DMA path)
  jfT  = transpose(jf_shard)            (PE transposes, copy-cast to bf16)
  confT = transpose(conf)               (PE transposes, copy-cast to bf16)
  q    = jfT.T-tiles @ Wq  -> qT        (bf16 MMs + PE transposes)
  kT   = Wk.T-tiles @ confT             (bf16 MMs, accumulated over D)
  s    = qT.T-tiles @ kT                (bf16 MMs, accumulated over P)
  E, denom = exp(s/32), fused row-sum   (ACT; no max-subtraction: |s|/32 < ~6)
  ET   = transpose(E) * prior           (PE transpose + DVE mul)
  gz   = ET.T-tiles @ conf              (bf16 MMs, accumulated over K)
  out  = gz * (1/denom) + jf            (fused DVE scalar_tensor_tensor, fp32)
"""

import numpy as np

B, K, D, P = 1024, 256, 1024, 768
N_CORES = 8
BS = B // N_CORES  # 128 rows per core

_COMPILED = {}


def _build():
    import concourse.mybir as mybir
    import concourse.tile as tile
    from concourse import bacc
    from concourse.compiler_utils import get_compiler_flags, set_compiler_flags
    from concourse.masks import make_identity

    flags = get_compiler_flags()
    if flags:
        set_compiler_flags(
            [
                f.replace("--enable-ldw-opt=false", "--enable-ldw-opt=true")
                for f in flags
            ]
        )

    F32 = mybir.dt.float32
    BF = mybir.dt.bfloat16
    KD = D // 128  # 8 contraction tiles over D
    MP = P // 128  # 6 partition tiles over P
    KT = K // 128  # 2 tiles over K

    nc = bacc.Bacc(
        "TRN2",
        target_bir_lowering=False,
        debug=False,
        num_devices=N_CORES,
    )

    jf = nc.dram_tensor("jf", [BS, D], F32, kind="ExternalInput")
    conf = nc.dram_tensor("conf", [K, D], F32, kind="ExternalInput")
    prior = nc.dram_tensor("prior", [K, 1], F32, kind="ExternalInput")
    wq = nc.dram_tensor("wq", [D, P], F32, kind="ExternalInput")
    wk = nc.dram_tensor("wk", [D, P], F32, kind="ExternalInput")
    out = nc.dram_tensor("out", [BS, D], F32, kind="ExternalOutput")

    with tile.TileContext(nc) as tc:
        with (
            tc.tile_pool(name="cst", bufs=1) as cst,
            tc.tile_pool(name="per", bufs=1) as per,
            tc.tile_pool(name="wqp", bufs=1) as wqp,
            tc.tile_pool(name="wkp", bufs=1) as wkp,
            tc.tile_pool(name="ps", bufs=6, space="PSUM") as ps,
            tc.tile_pool(name="pst", bufs=2, space="PSUM") as pst,
        ):
            # Identity first: its gpsimd memset/affine must precede the 16
            # SWDGE trigger instructions on the GpSimd queue, or it lands
            # ~13us late and stalls every transpose behind it.
            ident = cst.tile([128, 128], F32, tag="ident", name="ident")
            make_identity(nc, ident[:])
            ident_bf = cst.tile([128, 128], BF, tag="ident_bf", name="ident_bf")
            nc.vector.tensor_copy(ident_bf[:], ident[:])

            NQ = P // 2  # 384
            psq = [
                ps.tile([BS, 512], F32, tag="bank", name=f"psq{h}") for h in range(2)
            ]
            # PE warmup: ~4us of dummy matmuls to flip the HAM clock-gate to
            # 2.4 GHz before real matmuls arrive. Writes psq[0] (WAW-serialized
            # there, and q's first matmul has start=True so contents don't matter).
            with nc.named_scope("warmup"):
                for _ in range(40):
                    nc.tensor.matmul(
                        psq[0][:, 0:128], lhsT=ident_bf[:], rhs=ident_bf[:],
                        start=True, stop=True,
                    )

            # ---- W cast-DMAs: SWDGE triggers ahead of everything else so
            # all 16 SDMA engines saturate from t=0 (Wq chunks, then Wk).
            wqt = [
                wqp.tile([128, P], BF, tag=f"wq{kk}", name=f"wq{kk}")
                for kk in range(KD)
            ]
            wkt = [
                wkp.tile([128, P], BF, tag=f"wk{kk}", name=f"wk{kk}")
                for kk in range(KD)
            ]
            for kk in range(KD):
                nc.gpsimd.dma_start(
                    out=wqt[kk][:], in_=wq.ap()[128 * kk : 128 * (kk + 1), :]
                )
            for kk in range(KD):
                nc.gpsimd.dma_start(
                    out=wkt[kk][:], in_=wk.ap()[128 * kk : 128 * (kk + 1), :]
                )

            # ---- input DMAs (jf/conf/prior first, then Wq stream, Wk stream)
            jf_sb = per.tile([BS, D], F32, tag="jf", name="jf")
            nc.sync.dma_start(out=jf_sb[:], in_=jf.ap())
            conf_sb = [
                per.tile([128, D], F32, tag=f"conf{t}", name=f"conf{t}")
                for t in range(KT)
            ]
            for t in range(KT):
                nc.sync.dma_start(
                    out=conf_sb[t][:], in_=conf.ap()[128 * t : 128 * (t + 1), :]
                )
            prior_sb = per.tile([128, KT], F32, tag="prior", name="prior")
            nc.sync.dma_start(
                out=prior_sb[:],
                in_=prior.ap().rearrange("(t p) o -> p (t o)", p=128),
            )

            # conf as bf16 (rhs of the gz matmul)
            conf_bf = [
                per.tile([128, D], BF, tag=f"confbf{t}", name=f"confbf{t}")
                for t in range(KT)
            ]
            for t in range(KT):
                nc.vector.tensor_copy(conf_bf[t][:], conf_sb[t][:])

            # ---- jfT: bf16 PE transposes (1 cyc/row) from jf_bf
            jf_bf = per.tile([BS, D], BF, tag="jf_bf", name="jf_bf")
            jfT_t = [
                per.tile([128, BS], BF, tag=f"jfT{kk}", name=f"jfT{kk}")
                for kk in range(KD)
            ]
            with nc.named_scope("jfT"):
                nc.vector.tensor_copy(jf_bf[:], jf_sb[:])
                for kk in range(KD):
                    pt = pst.tile([128, 256], BF, tag="pc", name="pt")
                    nc.tensor.transpose(
                        pt[:, 0:128], jf_bf[:, 128 * kk : 128 * (kk + 1)], ident_bf[:]
                    )
                    nc.vector.tensor_copy(jfT_t[kk][:], pt[:, 0:128])
            jfT = [t[:] for t in jfT_t]

            # ---- confT: bf16 PE transposes from conf_bf
            confT_t = [
                per.tile([128, K], BF, tag=f"confT{kk}", name=f"confT{kk}")
                for kk in range(KD)
            ]
            with nc.named_scope("confT"):
                for kk in range(KD):
                    pc = pst.tile([128, 256], BF, tag="pc", name="pc")
                    for t in range(KT):
                        nc.tensor.transpose(
                            pc[:, 128 * t : 128 * (t + 1)],
                            conf_bf[t][:, 128 * kk : 128 * (kk + 1)],
                            ident_bf[:],
                        )
                    nc.vector.tensor_copy(confT_t[kk][:], pc[:])
            confT = [t[:] for t in confT_t]

            # ---- q = jf @ Wq : psum [BS, P] as two 384-wide banks
            with nc.named_scope("q_mm"):
                for kk in range(KD):
                    for h in range(2):
                        nc.tensor.matmul(
                            psq[h][:, 0:NQ],
                            lhsT=jfT[kk],
                            rhs=wqt[kk][:, NQ * h : NQ * (h + 1)],
                            start=(kk == 0),
                            stop=(kk == KD - 1),
                        )

            # ---- qT: psum -> bf16, then bf16 PE transposes
            q_bf = per.tile([BS, P], BF, tag="q_bf", name="q_bf")
            qT_t = [
                per.tile([128, BS], BF, tag=f"qT{pp}", name=f"qT{pp}")
                for pp in range(MP)
            ]
            with nc.named_scope("qT"):
                for h in range(2):
                    nc.vector.tensor_copy(q_bf[:, NQ * h : NQ * (h + 1)], psq[h][:, 0:NQ])
                for pp in range(MP):
                    pt = pst.tile([128, 256], BF, tag="pc", name="pt")
                    nc.tensor.transpose(
                        pt[:, 0:128], q_bf[:, 128 * pp : 128 * (pp + 1)], ident_bf[:]
                    )
                    nc.vector.tensor_copy(qT_t[pp][:], pt[:, 0:128])
            qT = [t[:] for t in qT_t]

            # ---- kT = (conf @ Wk).T : 6 m-tiles [128, K], packed 2 per bank
            psk = [
                ps.tile([128, 512], F32, tag="bank", name=f"psk{i}")
                for i in range(MP)
            ]

            def psk_ap(mm):
                return psk[mm][:, 0:K]

            with nc.named_scope("kT_mm"):
                for kk in range(KD):
                    for mm in range(MP):
                        nc.tensor.matmul(
                            psk_ap(mm),
                            lhsT=wkt[kk][:, 128 * mm : 128 * (mm + 1)],
                            rhs=confT[kk],
                            start=(kk == 0),
                            stop=(kk == KD - 1),
                        )

            kT = [
                per.tile([128, K], BF, tag=f"kT{mm}", name=f"kT{mm}")
                for mm in range(MP)
            ]
            with nc.named_scope("kT_copy"):
                for mm in range(MP):
                    nc.vector.tensor_copy(kT[mm][:], psk_ap(mm))

            # ---- scores = q @ k.T (accumulate over P tiles)
            ps_s = ps.tile([BS, 512], F32, tag="bank", name="ps_s")[:, 0:K]
            with nc.named_scope("scores"):
                for pp in range(MP):
                    nc.tensor.matmul(
                        ps_s[:],
                        lhsT=qT[pp],
                        rhs=kT[pp][:],
                        start=(pp == 0),
                        stop=(pp == MP - 1),
                    )

            # ---- softmax numerator + denominator (no max-subtraction)
            E_sb = per.tile([BS, K], BF, tag="E", name="E")
            denom = per.tile([BS, 1], F32, tag="denom", name="denom")
            r_sb = per.tile([BS, 1], F32, tag="r", name="r")
            with nc.named_scope("softmax"):
                nc.scalar.activation(
                    E_sb[:],
                    ps_s[:],
                    mybir.ActivationFunctionType.Exp,
                    scale=1.0 / 32.0,
                    accum_out=denom[:],
                )
                nc.vector.reciprocal(r_sb[:], denom[:])

            # ---- ET = E.T * prior  -> 2 bf16 tiles [128, BS]
            ET = [
                per.tile([128, BS], BF, tag=f"ET{t}", name=f"ET{t}") for t in range(KT)
            ]
            with nc.named_scope("ET"):
                for t in range(KT):
                    pa = pst.tile([128, 128], BF, tag="pc", name="pa")
                    nc.tensor.transpose(
                        pa[:], E_sb[:, 128 * t : 128 * (t + 1)], ident_bf[:]
                    )
                    nc.vector.tensor_scalar_mul(
                        ET[t][:], pa[:], prior_sb[:, t : t + 1]
                    )

            # ---- gz = E @ (conf * prior) : psum [BS, D] as two 512-banks
            ND = D // 2  # 512
            psg = [
                ps.tile([BS, ND], F32, tag="bank", name=f"psg{h}") for h in range(2)
            ]
            with nc.named_scope("gz_mm"):
                for t in range(KT):
                    for h in range(2):
                        nc.tensor.matmul(
                            psg[h][:],
                            lhsT=ET[t][:],
                            rhs=conf_bf[t][:, ND * h : ND * (h + 1)],
                            start=(t == 0),
                            stop=(t == KT - 1),
                        )

            # ---- out = gz * (1/denom) + jf ; 4-way split so the fused
            # multiply-add, and the output DMAs pipeline.
            out_sb = [
                per.tile([BS, ND], F32, tag=f"out{h}", name=f"out{h}")
                for h in range(2)
            ]
            NE = ND // 2  # 256
            with nc.named_scope("epilogue"):
                for qtr in range(4):
                    h, j = qtr // 2, qtr % 2
                    nc.vector.scalar_tensor_tensor(
                        out_sb[h][:, NE * j : NE * (j + 1)],
                        psg[h][:, NE * j : NE * (j + 1)],
                        r_sb[:],
                        jf_sb[:, ND * h + NE * j : ND * h + NE * (j + 1)],
                        op0=mybir.AluOpType.mult,
                        op1=mybir.AluOpType.add,
                    )
                    nc.sync.dma_start(
                        out=out.ap()[:, ND * h + NE * j : ND * h + NE * (j + 1)],
                        in_=out_sb[h][:, NE * j : NE * (j + 1)],
                    )

    nc.compile()
    return nc


def _get_compiled():
    if "nc" not in _COMPILED:
        _COMPILED["nc"] = _build()
    return _COMPILED["nc"]


def kernel(joint_feature, confounder_dictionary, prior, Wq, Wk):
    from concourse import bass_utils

    nc = _get_compiled()

    jf = np.ascontiguousarray(np.asarray(joint_feature, dtype=np.float32))
    conf = np.ascontiguousarray(np.asarray(confounder_dictionary, dtype=np.float32))
    pri = np.ascontiguousarray(np.asarray(prior, dtype=np.float32))
    wq = np.ascontiguousarray(np.asarray(Wq, dtype=np.float32))
    wk = np.ascontiguousarray(np.asarray(Wk, dtype=np.float32))

    in_maps = [
        {
            "jf": jf[i * BS : (i + 1) * BS],
            "conf": conf,
            "prior": pri,
            "wq": wq,
            "wk": wk,
        }
        for i in range(N_CORES)
    ]

    res = bass_utils.run_bass_kernel_spmd(
        nc, in_maps, core_ids=list(range(N_CORES))
    )
    return np.concatenate([res.results[i]["out"] for i in range(N_CORES)], axis=0)


# revision 13
# speedup vs baseline: 1.0196x; 1.0196x over previous
"""Trainium2 Bass kernel for nn_CCIM (dot-product intervention / CCIM block).

Reference computation (B=1024, K=256, D=1024, P=768):
    q = jf @ Wq                      [B, P]
    k = conf @ Wk                    [K, P]
    s = (q @ k.T) / 32               [B, K]
    a = softmax(s, axis=-1)          [B, K]
    out = jf + a @ (conf * prior)    [B, D]

Distribution: data-parallel over B across 8 NeuronCores (128 rows each);
conf/prior/Wq/Wk replicated on every core; no collectives.

Per-core algorithm (matmuls in bf16 with fp32 PSUM accumulation; jf stays
exact fp32 through to the final add, so bf16 rounding only perturbs the small
g_z correction term):
  weights stream in as bf16 via SWDGE cast-DMA (cast happens in the DMA path)
  jfT  = transpose(jf_shard)            (PE transposes, copy-cast to bf16)
  confT = transpose(conf)               (PE transposes, copy-cast to bf16)
  q    = jfT.T-tiles @ Wq  -> qT        (bf16 MMs + PE transposes)
  kT   = Wk.T-tiles @ confT             (bf16 MMs, accumulated over D)
  s    = qT.T-tiles @ kT                (bf16 MMs, accumulated over P)
  E, denom = exp(s/32), fused row-sum   (ACT; no max-subtraction: |s|/32 < ~6)
  ET   = transpose(E) * prior           (PE transpose + DVE mul)
  gz   = ET.T-tiles @ conf              (bf16 MMs, accumulated over K)
  out  = gz * (1/denom) + jf            (fused DVE scalar_tensor_tensor, fp32)
"""

import numpy as np

B, K, D, P = 1024, 256, 1024, 768
N_CORES = 8
BS = B // N_CORES  # 128 rows per core

_COMPILED = {}


def _build():
    import concourse.mybir as mybir
    import concourse.tile as tile
    from concourse import bacc
    from concourse.compiler_utils import get_compiler_flags, set_compiler_flags
    from concourse.masks import make_identity

    flags = get_compiler_flags()
    if flags:
        set_compiler_flags(
            [
                f
                for f in flags
            ]
        )

    F32 = mybir.dt.float32
    BF = mybir.dt.bfloat16
    KD = D // 128  # 8 contraction tiles over D
    MP = P // 128  # 6 partition tiles over P
    KT = K // 128  # 2 tiles over K

    nc = bacc.Bacc(
        "TRN2",
        target_bir_lowering=False,
        debug=False,
        num_devices=N_CORES,
    )

    jf = nc.dram_tensor("jf", [BS, D], F32, kind="ExternalInput")
    conf = nc.dram_tensor("conf", [K, D], F32, kind="ExternalInput")
    prior = nc.dram_tensor("prior", [K, 1], F32, kind="ExternalInput")
    wq = nc.dram_tensor("wq", [D, P], F32, kind="ExternalInput")
    wk = nc.dram_tensor("wk", [D, P], F32, kind="ExternalInput")
    out = nc.dram_tensor("out", [BS, D], F32, kind="ExternalOutput")

    with tile.TileContext(nc) as tc:
        with (
            tc.tile_pool(name="cst", bufs=1) as cst,
            tc.tile_pool(name="per", bufs=1) as per,
            tc.tile_pool(name="wqp", bufs=1) as wqp,
            tc.tile_pool(name="wkp", bufs=1) as wkp,
            tc.tile_pool(name="ps", bufs=6, space="PSUM") as ps,
            tc.tile_pool(name="pst", bufs=2, space="PSUM") as pst,
        ):
            # Identity first: its gpsimd memset/affine must precede the 16
            # SWDGE trigger instructions on the GpSimd queue, or it lands
            # ~13us late and stalls every transpose behind it.
            ident = cst.tile([128, 128], F32, tag="ident", name="ident")
            make_identity(nc, ident[:])
            ident_bf = cst.tile([128, 128], BF, tag="ident_bf", name="ident_bf")
            nc.vector.tensor_copy(ident_bf[:], ident[:])

            NQ = P // 2  # 384
            psq = [
                ps.tile([BS, 512], F32, tag="bank", name=f"psq{h}") for h in range(2)
            ]
            # PE warmup: ~4us of dummy matmuls to flip the HAM clock-gate to
            # 2.4 GHz before real matmuls arrive. Writes psq[0] (WAW-serialized
            # there, and q's first matmul has start=True so contents don't matter).
            with nc.named_scope("warmup"):
                for _ in range(40):
                    nc.tensor.matmul(
                        psq[0][:, 0:128], lhsT=ident_bf[:], rhs=ident_bf[:],
                        start=True, stop=True,
                    )

            # ---- W cast-DMAs: SWDGE triggers ahead of everything else so
            # all 16 SDMA engines saturate from t=0 (Wq chunks, then Wk).
            wqt = [
                wqp.tile([128, P], BF, tag=f"wq{kk}", name=f"wq{kk}")
                for kk in range(KD)
            ]
            wkt = [
                wkp.tile([128, P], BF, tag=f"wk{kk}", name=f"wk{kk}")
                for kk in range(KD)
            ]
            for kk in range(KD):
                nc.gpsimd.dma_start(
                    out=wqt[kk][:], in_=wq.ap()[128 * kk : 128 * (kk + 1), :]
                )
            for kk in range(KD):
                nc.gpsimd.dma_start(
                    out=wkt[kk][:], in_=wk.ap()[128 * kk : 128 * (kk + 1), :]
                )

            # ---- input DMAs (jf/conf/prior first, then Wq stream, Wk stream)
            jf_sb = per.tile([BS, D], F32, tag="jf", name="jf")
            nc.sync.dma_start(out=jf_sb[:], in_=jf.ap())
            conf_sb = [
                per.tile([128, D], F32, tag=f"conf{t}", name=f"conf{t}")
                for t in range(KT)
            ]
            for t in range(KT):
                nc.sync.dma_start(
                    out=conf_sb[t][:], in_=conf.ap()[128 * t : 128 * (t + 1), :]
                )
            prior_sb = per.tile([128, KT], F32, tag="prior", name="prior")
            nc.sync.dma_start(
                out=prior_sb[:],
                in_=prior.ap().rearrange("(t p) o -> p (t o)", p=128),
            )

            # conf as bf16 (rhs of the gz matmul)
            conf_bf = [
                per.tile([128, D], BF, tag=f"confbf{t}", name=f"confbf{t}")
                for t in range(KT)
            ]
            for t in range(KT):
                nc.vector.tensor_copy(conf_bf[t][:], conf_sb[t][:])

            # ---- jfT: bf16 PE transposes (1 cyc/row) from jf_bf
            jf_bf = per.tile([BS, D], BF, tag="jf_bf", name="jf_bf")
            jfT_t = [
                per.tile([128, BS], BF, tag=f"jfT{kk}", name=f"jfT{kk}")
                for kk in range(KD)
            ]
            with nc.named_scope("jfT"):
                nc.vector.tensor_copy(jf_bf[:], jf_sb[:])
                for kk in range(KD):
                    pt = pst.tile([128, 256], BF, tag="pc", name="pt")
                    nc.tensor.transpose(
                        pt[:, 0:128], jf_bf[:, 128 * kk : 128 * (kk + 1)], ident_bf[:]
                    )
                    nc.vector.tensor_copy(jfT_t[kk][:], pt[:, 0:128])
            jfT = [t[:] for t in jfT_t]

            # ---- confT: bf16 PE transposes from conf_bf
            confT_t = [
                per.tile([128, K], BF, tag=f"confT{kk}", name=f"confT{kk}")
                for kk in range(KD)
            ]
            with nc.named_scope("confT"):
                for kk in range(KD):
                    pc = pst.tile([128, 256], BF, tag="pc", name="pc")
                    for t in range(KT):
                        nc.tensor.transpose(
                            pc[:, 128 * t : 128 * (t + 1)],
                            conf_bf[t][:, 128 * kk : 128 * (kk + 1)],
                            ident_bf[:],
                        )
                    nc.vector.tensor_copy(confT_t[kk][:], pc[:])
            confT = [t[:] for t in confT_t]

            # ---- q = jf @ Wq : psum [BS, P] as two 384-wide banks
            with nc.named_scope("q_mm"):
                for kk in range(KD):
                    for h in range(2):
                        nc.tensor.matmul(
                            psq[h][:, 0:NQ],
                            lhsT=jfT[kk],
                            rhs=wqt[kk][:, NQ * h : NQ * (h + 1)],
                            start=(kk == 0),
                            stop=(kk == KD - 1),
                        )

            # ---- qT: psum -> bf16, then bf16 PE transposes
            q_bf = per.tile([BS, P], BF, tag="q_bf", name="q_bf")
            qT_t = [
                per.tile([128, BS], BF, tag=f"qT{pp}", name=f"qT{pp}")
                for pp in range(MP)
            ]
            with nc.named_scope("qT"):
                for h in range(2):
                    nc.vector.tensor_copy(q_bf[:, NQ * h : NQ * (h + 1)], psq[h][:, 0:NQ])
                for pp in range(MP):
                    pt = pst.tile([128, 256], BF, tag="pc", name="pt")
                    nc.tensor.transpose(
                        pt[:, 0:128], q_bf[:, 128 * pp : 128 * (pp + 1)], ident_bf[:]
                    )
                    nc.vector.tensor_copy(qT_t[pp][:], pt[:, 0:128])
            qT = [t[:] for t in qT_t]

            # ---- kT = (conf @ Wk).T : 6 m-tiles [128, K], packed 2 per bank
            psk = [
                ps.tile([128, 512], F32, tag="bank", name=f"psk{i}")
                for i in range(MP)
            ]

            def psk_ap(mm):
                return psk[mm][:, 0:K]

            with nc.named_scope("kT_mm"):
                for kk in range(KD):
                    for mm in range(MP):
                        nc.tensor.matmul(
                            psk_ap(mm),
                            lhsT=wkt[kk][:, 128 * mm : 128 * (mm + 1)],
                            rhs=confT[kk],
                            start=(kk == 0),
                            stop=(kk == KD - 1),
                        )

            kT = [
                per.tile([128, K], BF, tag=f"kT{mm}", name=f"kT{mm}")
                for mm in range(MP)
            ]
            with nc.named_scope("kT_copy"):
                for mm in range(MP):
                    nc.vector.tensor_copy(kT[mm][:], psk_ap(mm))

            # ---- scores = q @ k.T (accumulate over P tiles)
            ps_s = ps.tile([BS, 512], F32, tag="bank", name="ps_s")[:, 0:K]
            with nc.named_scope("scores"):
                for pp in range(MP):
                    nc.tensor.matmul(
                        ps_s[:],
                        lhsT=qT[pp],
                        rhs=kT[pp][:],
                        start=(pp == 0),
                        stop=(pp == MP - 1),
                    )

            # ---- softmax numerator + denominator (no max-subtraction)
            E_sb = per.tile([BS, K], BF, tag="E", name="E")
            denom = per.tile([BS, 1], F32, tag="denom", name="denom")
            r_sb = per.tile([BS, 1], F32, tag="r", name="r")
            with nc.named_scope("softmax"):
                nc.scalar.activation(
                    E_sb[:],
                    ps_s[:],
                    mybir.ActivationFunctionType.Exp,
                    scale=1.0 / 32.0,
                    accum_out=denom[:],
                )
                nc.vector.reciprocal(r_sb[:], denom[:])

            # ---- ET = E.T * prior  -> 2 bf16 tiles [128, BS]
            ET = [
                per.tile([128, BS], BF, tag=f"ET{t}", name=f"ET{t}") for t in range(KT)
            ]
            with nc.named_scope("ET"):
                for t in range(KT):
                    pa = pst.tile([128, 128], BF, tag="pc", name="pa")
                    nc.tensor.transpose(
                        pa[:], E_sb[:, 128 * t : 128 * (t + 1)], ident_bf[:]
                    )
                    nc.vector.tensor_scalar_mul(
                        ET[t][:], pa[:], prior_sb[:, t : t + 1]
                    )

            # ---- gz = E @ (conf * prior) : psum [BS, D] as two 512-banks
            ND = D // 2  # 512
            psg = [
                ps.tile([BS, ND], F32, tag="bank", name=f"psg{h}") for h in range(2)
            ]
            with nc.named_scope("gz_mm"):
                for t in range(KT):
                    for h in range(2):
                        nc.tensor.matmul(
                            psg[h][:],
                            lhsT=ET[t][:],
                            rhs=conf_bf[t][:, ND * h : ND * (h + 1)],
                            start=(t == 0),
                            stop=(t == KT - 1),
                        )

            # ---- out = gz * (1/denom) + jf ; 4-way split so the fused
            # multiply-add, and the output DMAs pipeline.
            out_sb = [
                per.tile([BS, ND], F32, tag=f"out{h}", name=f"out{h}")
                for h in range(2)
            ]
            NE = ND // 2  # 256
            with nc.named_scope("epilogue"):
                for qtr in range(4):
                    h, j = qtr // 2, qtr % 2
                    nc.vector.scalar_tensor_tensor(
                        out_sb[h][:, NE * j : NE * (j + 1)],
                        psg[h][:, NE * j : NE * (j + 1)],
                        r_sb[:],
                        jf_sb[:, ND * h + NE * j : ND * h + NE * (j + 1)],
                        op0=mybir.AluOpType.mult,
                        op1=mybir.AluOpType.add,
                    )
                    nc.sync.dma_start(
                        out=out.ap()[:, ND * h + NE * j : ND * h + NE * (j + 1)],
                        in_=out_sb[h][:, NE * j : NE * (j + 1)],
                    )

    nc.compile()
    return nc


def _get_compiled():
    if "nc" not in _COMPILED:
        _COMPILED["nc"] = _build()
    return _COMPILED["nc"]


def kernel(joint_feature, confounder_dictionary, prior, Wq, Wk):
    from concourse import bass_utils

    nc = _get_compiled()

    jf = np.ascontiguousarray(np.asarray(joint_feature, dtype=np.float32))
    conf = np.ascontiguousarray(np.asarray(confounder_dictionary, dtype=np.float32))
    pri = np.ascontiguousarray(np.asarray(prior, dtype=np.float32))
    wq = np.ascontiguousarray(np.asarray(Wq, dtype=np.float32))
    wk = np.ascontiguousarray(np.asarray(Wk, dtype=np.float32))

    in_maps = [
        {
            "jf": jf[i * BS : (i + 1) * BS],
            "conf": conf,
            "prior": pri,
            "wq": wq,
            "wk": wk,
        }
        for i in range(N_CORES)
    ]

    res = bass_utils.run_bass_kernel_spmd(
        nc, in_maps, core_ids=list(range(N_CORES))
    )
    return np.concatenate([res.results[i]["out"] for i in range(N_CORES)], axis=0)
